# revision 8
# baseline (speedup 1.0000x reference)
"""DeepPoly ReLU abstract-transformer kernel for 8 TRN2 NeuronCores.

Reference semantics (elementwise over N = 16,777,216):
    x_out     = relu(x)
    neg  = upper <= 0          -> bounds (0, 0)
    pos  = lower >= 0          -> bounds (upper, upper)
    crossing   (else)          -> (lower, upper^2 / (upper - lower))

Memory-bound problem; harness tolerance is rel_err < 2e-2, which leaves room
to move all six DRAM tensors in bf16 (worst-case rounding ~4e-3) and halve
HBM traffic: 12 B/elem instead of 24.  The host converts f32 -> bf16 before
upload and bf16 -> f32 after download; the sensitive arithmetic (denominator,
reciprocal) runs in f32 on-chip.

Device formulation per tile (inputs x, l, u in bf16):
    x_out = relu(x)             # scalar ACT, in place
    up    = relu(u)             # scalar ACT (exact)
    nl    = relu(-l)            # scalar ACT (exact)
    sq    = up^2                # scalar ACT Square
    d     = up + nl             # DVE (f32, exact)
    r     = 1/d                 # DVE reciprocal_approx_fast (f32)
    uo    = sq * r              # DVE -> bf16
      neg: 0*(1/-l)=0; pos: u^2/u=u; crossing: u^2/(u-l)
    pp    = (l >= 0) mask       # DVE ts
    le    = (u <= 0) mask       # DVE ts
    lower_out (in place on l):
      where(le) <- 0            # DVE copy_predicated
      where(pp) <- u            # DVE copy_predicated (exact)

gpsimd is deliberately idle: its Q7 ucode runs bf16 tensor ops ~10x slower
than f32 (HW-measured 673us vs 115us for this kernel).

io="packed": x/l/u are interleaved per tile in ONE DRAM tensor
[P, ntiles, 3, tile_f] (same for outputs), so each tile moves with a single
3 MiB DMA with 24 KB contiguous per-partition lines, and upper_out is
written back into the u slot of the same SBUF tile.

Sharding: pure elementwise -> split N across the 8 cores; each core sees
2,097,152 elements. No communication.
"""

import numpy as np
import ml_dtypes

import concourse.bacc as bacc
import concourse.mybir as mybir
import concourse.tile as tile
from concourse import bass_utils

N_CORES = 8
N_TOTAL = 16777216
P = 128
NCOLS = N_TOTAL // N_CORES // P  # 16384

# default (current best) config; build_nc kwargs override for A/B tests
TILE_F = 4096
BUFS = 2
IO = "packed"
MASK_DT = "u16"
SQ_DT = "f32"
IN_DMA = "sync"
OUT_DMA = "scalar"
STAGGERED = False

_BF16 = mybir.dt.bfloat16
_F32 = mybir.dt.float32
_RELU = mybir.ActivationFunctionType.Relu
_SQUARE = mybir.ActivationFunctionType.Square
_OP = mybir.AluOpType

IN_NAMES = ("x", "lower", "upper")
OUT_NAMES = ("x_out", "lower_out", "upper_out")


def build_nc(
    ncols: int = NCOLS,
    tile_f: int = TILE_F,
    bufs: int = BUFS,
    reps: int = 1,
    io: str = IO,
    mask_dt: str = MASK_DT,
    sq_dt: str = SQ_DT,
    in_dma: str = IN_DMA,
    out_dma: str = OUT_DMA,
    staggered: bool = STAGGERED,
):
    assert ncols % tile_f == 0
    ntiles = ncols // tile_f
    nc = bacc.Bacc(
        "TRN2", target_bir_lowering=False, debug=False, num_devices=N_CORES
    )
    _MASK = mybir.dt.uint16 if mask_dt == "u16" else mybir.dt.uint8
    _SQ = _F32 if sq_dt == "f32" else _BF16

    if io == "packed":
        pin = nc.dram_tensor(
            "pin", [P, ntiles, 3, tile_f], _BF16, kind="ExternalInput"
        ).ap()
        pout = nc.dram_tensor(
            "pout", [P, ntiles, 3, tile_f], _BF16, kind="ExternalOutput"
        ).ap()
    else:
        ins = [
            nc.dram_tensor(n, [P, ncols], _BF16, kind="ExternalInput").ap()
            for n in IN_NAMES
        ]
        outs = [
            nc.dram_tensor(n, [P, ncols], _BF16, kind="ExternalOutput").ap()
            for n in OUT_NAMES
        ]

    ieng = lambda: getattr(nc, in_dma)
    oeng = lambda: getattr(nc, out_dma)

    with tile.TileContext(nc) as tc:
        with (
            tc.tile_pool(name="const", bufs=1) as cpool,
            tc.tile_pool(name="io", bufs=bufs) as pool,
        ):
            zt = cpool.tile([P, tile_f], _BF16, tag="zero")
            nc.gpsimd.memset(zt[:], 0.0)

            def compute(xs, ls, us, i):
                """xs/ls/us: [P, tile_f] bf16 APs. Returns upper_out AP
                (uot) -- xs/ls are updated in place to x_out/lower_out."""
                nc.scalar.activation(xs, xs, _RELU)  # x_out, in place
                upt = pool.tile([P, tile_f], _BF16, tag="up")
                nc.scalar.activation(upt[:], us, _RELU)
                nlt = pool.tile([P, tile_f], _BF16, tag="nl")
                nc.scalar.activation(nlt[:], ls, _RELU, scale=-1.0)
                sqt = pool.tile([P, tile_f], _SQ, tag="sq")
                nc.scalar.activation(sqt[:], upt[:], _SQUARE)

                ppt = pool.tile([P, tile_f], _MASK, tag="pp")
                nc.vector.tensor_scalar(
                    out=ppt[:], in0=ls, scalar1=0.0, scalar2=None,
                    op0=_OP.is_ge,
                )
                let = pool.tile([P, tile_f], _MASK, tag="le")
                nc.vector.tensor_scalar(
                    out=let[:], in0=us, scalar1=0.0, scalar2=None,
                    op0=_OP.is_le,
                )
                dt = pool.tile([P, tile_f], _F32, tag="d")
                nc.vector.tensor_add(out=dt[:], in0=upt[:], in1=nlt[:])
                nc.vector.reciprocal_approx_fast(out=dt[:], in_=dt[:])

                nc.vector.copy_predicated(out=ls, mask=let[:], data=zt[:])
                nc.vector.copy_predicated(out=ls, mask=ppt[:], data=us)

                return sqt, dt

            def one_iter_packed(i):
                big = pool.tile([P, 3, tile_f], _BF16, tag="big")
                ieng().dma_start(out=big[:], in_=pin[:, i])
                xs, ls, us = big[:, 0], big[:, 1], big[:, 2]
                sqt, dt = compute(xs, ls, us, i)
                # upper_out overwrites the u slot (after all readers of u)
                nc.vector.tensor_mul(out=us, in0=sqt[:], in1=dt[:])
                oeng().dma_start(out=pout[:, i], in_=big[:])

            def one_iter_split(i):
                xt = pool.tile([P, tile_f], _BF16, tag="x")
                lt = pool.tile([P, tile_f], _BF16, tag="l")
                ut = pool.tile([P, tile_f], _BF16, tag="u")
                sl = lambda t: t[:, i * tile_f : (i + 1) * tile_f]
                ieng().dma_start(out=xt[:], in_=sl(ins[0]))
                ieng().dma_start(out=lt[:], in_=sl(ins[1]))
                ieng().dma_start(out=ut[:], in_=sl(ins[2]))
                sqt, dt = compute(xt[:], lt[:], ut[:], i)
                uot = pool.tile([P, tile_f], _BF16, tag="uo")
                nc.vector.tensor_mul(out=uot[:], in0=sqt[:], in1=dt[:])
                oeng().dma_start(out=sl(outs[0]), in_=xt[:])
                oeng().dma_start(out=sl(outs[1]), in_=lt[:])
                oeng().dma_start(out=sl(outs[2]), in_=uot[:])

            one_iter = one_iter_packed if io == "packed" else one_iter_split

            def body():
                for i in range(ntiles):
                    one_iter(i)

            if reps == 1:
                body()
            else:
                # benchmarking only: hardware loop keeps the body IRAM-resident
                with tc.For_i(0, reps, 1, staggered_reset=staggered):
                    body()
    nc.compile()
    return nc


def host_in_arrays(inputs: dict, io: str = IO, tile_f: int = TILE_F):
    """f32 host arrays -> {dram_name: [ncore, *core_shape] bf16}."""
    ntiles = NCOLS // tile_f
    cast = {}
    for k in IN_NAMES:
        a = np.asarray(inputs[k], dtype=np.float32)
        cast[k] = np.ascontiguousarray(a).astype(ml_dtypes.bfloat16)
    if io == "packed":
        # element n -> (core, p, i, f); tensors interleave on axis 3
        parts = [
            cast[k].reshape(N_CORES, P, ntiles, tile_f) for k in IN_NAMES
        ]
        packed = np.stack(parts, axis=3)  # [ncore, P, ntiles, 3, tile_f]
        return {"pin": np.ascontiguousarray(packed)}
    return {k: cast[k].reshape(N_CORES, P, NCOLS) for k in IN_NAMES}


def host_out_assemble(
    results: dict, io: str = IO, tile_f: int = TILE_F
):
    """{dram_name: [ncore, *core_shape]} -> tuple of 3 f32 (1, N) arrays."""
    ntiles = NCOLS // tile_f
    outs = []
    if io == "packed":
        pout = results["pout"]  # [ncore, P, ntiles, 3, tile_f]
        for t in range(3):
            a = pout[:, :, :, t, :].reshape(1, N_TOTAL)
            outs.append(a.astype(np.float32))
    else:
        for n in OUT_NAMES:
            outs.append(results[n].reshape(1, N_TOTAL).astype(np.float32))
    return tuple(outs)


def run(inputs: dict, trace: bool = False):
    """Shard, execute on 8 cores, gather. Returns (outputs_tuple, results_obj)."""
    arrs = host_in_arrays(inputs)
    in_maps = [
        {k: arrs[k][c] for k in arrs} for c in range(N_CORES)
    ]
    nc = build_nc()
    res = bass_utils.run_bass_kernel_spmd(
        nc, in_maps, core_ids=list(range(N_CORES)), trace=trace
    )
    out_names = ["pout"] if IO == "packed" else list(OUT_NAMES)
    stacked = {
        name: np.stack([np.asarray(res.results[c][name]) for c in range(N_CORES)])
        for name in out_names
    }
    return host_out_assemble(stacked), res


def kernel(**inputs):
    outs, _ = run(inputs, trace=False)
    return outs


# revision 9
# speedup vs baseline: 1.1545x; 1.1545x over previous
"""DeepPoly ReLU abstract-transformer kernel for 8 TRN2 NeuronCores.

Reference semantics (elementwise over N = 16,777,216):
    x_out     = relu(x)
    neg  = upper <= 0          -> bounds (0, 0)
    pos  = lower >= 0          -> bounds (upper, upper)
    crossing   (else)          -> (lower, upper^2 / (upper - lower))

Memory-bound problem; harness tolerance is rel_err < 2e-2, which leaves room
to move all six DRAM tensors in bf16 (worst-case rounding ~4e-3) and halve
HBM traffic: 12 B/elem instead of 24.  The host converts f32 -> bf16 before
upload and bf16 -> f32 after download; the sensitive arithmetic (denominator,
reciprocal) runs in f32 on-chip.

Device formulation per tile (inputs x, l, u in bf16):
    x_out = relu(x)             # scalar ACT, in place
    up    = relu(u)             # scalar ACT (exact)
    nl    = relu(-l)            # scalar ACT (exact)
    sq    = up^2                # scalar ACT Square
    d     = up + nl             # DVE (f32, exact)
    r     = 1/d                 # DVE reciprocal_approx_fast (f32)
    uo    = sq * r              # DVE -> bf16
      neg: 0*(1/-l)=0; pos: u^2/u=u; crossing: u^2/(u-l)
    pp    = (l >= 0) mask       # DVE ts
    le    = (u <= 0) mask       # DVE ts
    lower_out (in place on l):
      where(le) <- 0            # DVE copy_predicated
      where(pp) <- u            # DVE copy_predicated (exact)

gpsimd is deliberately idle: its Q7 ucode runs bf16 tensor ops ~10x slower
than f32 (HW-measured 673us vs 115us for this kernel).

io="packed": x/l/u are interleaved per tile in ONE DRAM tensor
[P, ntiles, 3, tile_f] (same for outputs), so each tile moves with a single
3 MiB DMA with 24 KB contiguous per-partition lines, and upper_out is
written back into the u slot of the same SBUF tile.

Sharding: pure elementwise -> split N across the 8 cores; each core sees
2,097,152 elements. No communication.
"""

import numpy as np
import ml_dtypes

import concourse.bacc as bacc
import concourse.mybir as mybir
import concourse.tile as tile
from concourse import bass_utils

N_CORES = 8
N_TOTAL = 16777216
P = 128
NCOLS = N_TOTAL // N_CORES // P  # 16384

# default (current best) config; build_nc kwargs override for A/B tests
TILE_F = 4096
BUFS = 2
IO = "packed"
MASK_DT = "u16"
SQ_DT = "f32"
IN_DMA = "sync"
OUT_DMA = "scalar"
STAGGERED = False

_BF16 = mybir.dt.bfloat16
_F32 = mybir.dt.float32
_RELU = mybir.ActivationFunctionType.Relu
_SQUARE = mybir.ActivationFunctionType.Square
_OP = mybir.AluOpType

IN_NAMES = ("x", "lower", "upper")
OUT_NAMES = ("x_out", "lower_out", "upper_out")


def build_nc(
    ncols: int = NCOLS,
    tile_f: int = TILE_F,
    bufs: int = BUFS,
    reps: int = 1,
    io: str = IO,
    mask_dt: str = MASK_DT,
    sq_dt: str = SQ_DT,
    in_dma: str = IN_DMA,
    out_dma: str = OUT_DMA,
    staggered: bool = STAGGERED,
    unroll: int = 1,
):
    assert ncols % tile_f == 0
    ntiles = ncols // tile_f
    nc = bacc.Bacc(
        "TRN2", target_bir_lowering=False, debug=False, num_devices=N_CORES
    )
    _MASK = mybir.dt.uint16 if mask_dt == "u16" else mybir.dt.uint8
    _SQ = _F32 if sq_dt == "f32" else _BF16

    if io == "packed":
        pin = nc.dram_tensor(
            "pin", [P, ntiles, 3, tile_f], _BF16, kind="ExternalInput"
        ).ap()
        pout = nc.dram_tensor(
            "pout", [P, ntiles, 3, tile_f], _BF16, kind="ExternalOutput"
        ).ap()
    else:
        ins = [
            nc.dram_tensor(n, [P, ncols], _BF16, kind="ExternalInput").ap()
            for n in IN_NAMES
        ]
        outs = [
            nc.dram_tensor(n, [P, ncols], _BF16, kind="ExternalOutput").ap()
            for n in OUT_NAMES
        ]

    ieng = lambda: getattr(nc, in_dma)
    oeng = lambda: getattr(nc, out_dma)

    with tile.TileContext(nc) as tc:
        with (
            tc.tile_pool(name="const", bufs=1) as cpool,
            tc.tile_pool(name="io", bufs=bufs) as pool,
        ):
            zt = cpool.tile([P, tile_f], _BF16, tag="zero")
            nc.gpsimd.memset(zt[:], 0.0)

            def compute(xs, ls, us, i):
                """xs/ls/us: [P, tile_f] bf16 APs. Returns upper_out AP
                (uot) -- xs/ls are updated in place to x_out/lower_out."""
                nc.scalar.activation(xs, xs, _RELU)  # x_out, in place
                upt = pool.tile([P, tile_f], _BF16, tag="up")
                nc.scalar.activation(upt[:], us, _RELU)
                nlt = pool.tile([P, tile_f], _BF16, tag="nl")
                nc.scalar.activation(nlt[:], ls, _RELU, scale=-1.0)
                sqt = pool.tile([P, tile_f], _SQ, tag="sq")
                nc.scalar.activation(sqt[:], upt[:], _SQUARE)

                ppt = pool.tile([P, tile_f], _MASK, tag="pp")
                nc.vector.tensor_scalar(
                    out=ppt[:], in0=ls, scalar1=0.0, scalar2=None,
                    op0=_OP.is_ge,
                )
                let = pool.tile([P, tile_f], _MASK, tag="le")
                nc.vector.tensor_scalar(
                    out=let[:], in0=us, scalar1=0.0, scalar2=None,
                    op0=_OP.is_le,
                )
                dt = pool.tile([P, tile_f], _F32, tag="d")
                nc.vector.tensor_add(out=dt[:], in0=upt[:], in1=nlt[:])
                nc.vector.reciprocal_approx_fast(out=dt[:], in_=dt[:])

                nc.vector.copy_predicated(out=ls, mask=let[:], data=zt[:])
                nc.vector.copy_predicated(out=ls, mask=ppt[:], data=us)

                return sqt, dt

            def one_iter_packed(i):
                big = pool.tile([P, 3, tile_f], _BF16, tag="big")
                ieng().dma_start(out=big[:], in_=pin[:, i])
                xs, ls, us = big[:, 0], big[:, 1], big[:, 2]
                sqt, dt = compute(xs, ls, us, i)
                # upper_out overwrites the u slot (after all readers of u)
                nc.vector.tensor_mul(out=us, in0=sqt[:], in1=dt[:])
                oeng().dma_start(out=pout[:, i], in_=big[:])

            def one_iter_split(i):
                xt = pool.tile([P, tile_f], _BF16, tag="x")
                lt = pool.tile([P, tile_f], _BF16, tag="l")
                ut = pool.tile([P, tile_f], _BF16, tag="u")
                sl = lambda t: t[:, i * tile_f : (i + 1) * tile_f]
                ieng().dma_start(out=xt[:], in_=sl(ins[0]))
                ieng().dma_start(out=lt[:], in_=sl(ins[1]))
                ieng().dma_start(out=ut[:], in_=sl(ins[2]))
                sqt, dt = compute(xt[:], lt[:], ut[:], i)
                uot = pool.tile([P, tile_f], _BF16, tag="uo")
                nc.vector.tensor_mul(out=uot[:], in0=sqt[:], in1=dt[:])
                oeng().dma_start(out=sl(outs[0]), in_=xt[:])
                oeng().dma_start(out=sl(outs[1]), in_=lt[:])
                oeng().dma_start(out=sl(outs[2]), in_=uot[:])

            one_iter = one_iter_packed if io == "packed" else one_iter_split

            def body():
                for i in range(ntiles):
                    one_iter(i)

            if reps == 1:
                body()
            else:
                # benchmarking only: hardware loop keeps the body IRAM-resident.
                # unroll k: amortize the per-iteration all-engine barrier (and
                # its pipeline drain) over k full passes.
                assert reps % unroll == 0
                with tc.For_i(0, reps // unroll, 1, staggered_reset=staggered):
                    for _ in range(unroll):
                        body()
    nc.compile()
    return nc


def host_in_arrays(inputs: dict, io: str = IO, tile_f: int = TILE_F):
    """f32 host arrays -> {dram_name: [ncore, *core_shape] bf16}."""
    ntiles = NCOLS // tile_f
    cast = {}
    for k in IN_NAMES:
        a = np.asarray(inputs[k], dtype=np.float32)
        cast[k] = np.ascontiguousarray(a).astype(ml_dtypes.bfloat16)
    if io == "packed":
        # element n -> (core, p, i, f); tensors interleave on axis 3
        parts = [
            cast[k].reshape(N_CORES, P, ntiles, tile_f) for k in IN_NAMES
        ]
        packed = np.stack(parts, axis=3)  # [ncore, P, ntiles, 3, tile_f]
        return {"pin": np.ascontiguousarray(packed)}
    return {k: cast[k].reshape(N_CORES, P, NCOLS) for k in IN_NAMES}


def host_out_assemble(
    results: dict, io: str = IO, tile_f: int = TILE_F
):
    """{dram_name: [ncore, *core_shape]} -> tuple of 3 f32 (1, N) arrays."""
    ntiles = NCOLS // tile_f
    outs = []
    if io == "packed":
        pout = results["pout"]  # [ncore, P, ntiles, 3, tile_f]
        for t in range(3):
            a = pout[:, :, :, t, :].reshape(1, N_TOTAL)
            outs.append(a.astype(np.float32))
    else:
        for n in OUT_NAMES:
            outs.append(results[n].reshape(1, N_TOTAL).astype(np.float32))
    return tuple(outs)


def run(inputs: dict, trace: bool = False):
    """Shard, execute on 8 cores, gather. Returns (outputs_tuple, results_obj)."""
    arrs = host_in_arrays(inputs)
    in_maps = [
        {k: arrs[k][c] for k in arrs} for c in range(N_CORES)
    ]
    nc = build_nc()
    res = bass_utils.run_bass_kernel_spmd(
        nc, in_maps, core_ids=list(range(N_CORES)), trace=trace
    )
    out_names = ["pout"] if IO == "packed" else list(OUT_NAMES)
    stacked = {
        name: np.stack([np.asarray(res.results[c][name]) for c in range(N_CORES)])
        for name in out_names
    }
    return host_out_assemble(stacked), res


def kernel(**inputs):
    outs, _ = run(inputs, trace=False)
    return outs
